# revision 10
# baseline (speedup 1.0000x reference)
"""Trainium2 Bass kernel for the RNN auto-encoder problem.

Model (per batch row b):
  encoder: h_t = tanh(x_t * wih_e + Whh_e h_{t-1} + b_e),  h_0 = 0,  t = 1..T
  features = sigmoid(Wenc h_T + benc)                       (3 dims)
  decoder: h0 = Wdec features + bdec
           h_1 = tanh(Whh_d h0 + b_d)
           h_t = tanh(Wt h_{t-1} + bt),  Wt = Whh_d + wih_d wout^T (x feedback folded)
           out_t = wout . h_t + bout
  loss = masked MSE(input, out)

Sharding: data parallel, B=256 -> 32 rows on each of 8 cores; weights replicated.

Device design (per core, B=32):
  state hT  : SBUF [128, 8*32]  (partition = H row % 128, free = chunk*32 + b)
  weights   : W^T as rhs slabs [128, 8*1024] (free = kchunk*1024 + n)
  per step  : 8 accumulating fp32r matmuls (lhsT = hT chunk [128,32]
              stationary, rhs = W^T slab [128,512]) per N-half -> PSUM
              z [32,512]; PSUM->SBUF move (encoder: fused with the rank-1
              x_t*wih input term via one DVE scalar_tensor_tensor); 4 PE
              transposes -> PSUM zT [128,128]; tanh fused into the
              PSUM->SBUF copy with per-partition (per-H-row) bias.
  decoder out_t collected via 8 N=1 matmuls into a persistent PSUM bank.

All host-prepared constants ship in ONE dram blob -> ONE DMA -> one
semaphore, because the fp32r matmul weight-load path tolerates only a
single sync wait per instruction.
"""

import numpy as np

import concourse.bacc as bacc
import concourse.bass as bass
import concourse.mybir as mybir
import concourse.tile as tile
from concourse.bass_utils import run_bass_kernel_spmd

B, T, H, I = 256, 512, 1024, 1
NCORES = 8
BC = B // NCORES          # 32 batch rows per core
KC = H // 128             # 8 contraction chunks
F32 = mybir.dt.float32
F32R = mybir.dt.float32r

USE_F32R = True           # fp32r matmuls: 1 cyc/row at N>=512 (vs 4 for fp32)
MDT = F32R if USE_F32R else F32


def blob_layout(t_enc, t_dec):
    """name -> (parts, free_off, free_len); everything lives in one
    [128, total] fp32 blob."""
    spec = [
        ("wet", 128, KC * H), ("wdt", 128, KC * H), ("wdh", 128, KC * H),
        ("wenct", 128, KC * 4), ("woutt", 128, KC),
        ("bias_e", 128, KC), ("bias_d1", 128, KC), ("bias_dt", 128, KC),
        ("bias_dec", 128, KC),
        ("ident", BC, BC), ("wdect", 4, H), ("bencr", 1, 4),
        ("onesr", 1, BC), ("h0", 128, KC * BC), ("wihb", BC, H),
        ("xin", BC, t_enc), ("iota", BC, t_dec), ("seqf", BC, 1),
        ("bout", BC, 1),
    ]
    out, off = {}, 0
    for name, p, n in spec:
        out[name] = (p, off, n)
        off += n
    return out, off


def build(t_enc=T, t_dec=T):
    """Build the per-core Bass kernel. All 8 cores run the same program."""
    nc = bacc.Bacc("TRN2", target_bir_lowering=False)
    lay, total = blob_layout(t_enc, t_dec)

    d_blob = nc.dram_tensor("blob", [128, total], MDT, kind="ExternalInput")
    d_out = nc.dram_tensor("output", [BC, t_dec], F32, kind="ExternalOutput")
    d_feat = nc.dram_tensor("features", [BC, 3], MDT, kind="ExternalOutput")
    d_loss = nc.dram_tensor("losspart", [BC, 1], F32, kind="ExternalOutput")

    with tile.TileContext(nc) as tc:
        with (
            tc.tile_pool(name="singles", bufs=1) as singles,
            tc.tile_pool(name="s_z", bufs=3) as s_z,
            tc.tile_pool(name="s_hT", bufs=3) as s_hT,
            tc.tile_pool(name="zpool", bufs=2, space="PSUM") as zpool,
            tc.tile_pool(name="tpool", bufs=2, space="PSUM") as tpool,
            tc.tile_pool(name="xpool", bufs=1, space="PSUM") as xpool,
            tc.tile_pool(name="mpool", bufs=1, space="PSUM") as mpool,
        ):
            blob = singles.tile([128, total], MDT)
            nc.sync.dma_start(blob[:], d_blob[:])

            def sl(name, dtype=None):
                p, o, n = lay[name]
                ap = blob[:p, o:o + n]
                return ap.bitcast(dtype) if dtype is not None else ap

            wet, wdt, wdh = sl("wet"), sl("wdt"), sl("wdh")
            wenct, woutt = sl("wenct"), sl("woutt", F32)
            bias_e, bias_d1 = sl("bias_e", F32), sl("bias_d1", F32)
            bias_dt, bias_dec = sl("bias_dt", F32), sl("bias_dec", F32)
            ident, wdect, bencr, onesr = (sl("ident"), sl("wdect"),
                                          sl("bencr"), sl("onesr"))
            wihb, xin = sl("wihb", F32), sl("xin", F32)
            iota, seqf, bout = sl("iota", F32), sl("seqf", F32), sl("bout", F32)

            hT = sl("h0")   # initial hidden state (zeros from host)

            def step(hT_prev, w_rhs, bias_col, t_x=None):
                """One RNN step -> new hT tile.

                z = W h (+ x_t * wih on encoder); h' = tanh(z + bias)
                """
                hT_new = s_hT.tile([128, KC * BC], MDT)
                for half in range(2):
                    zp = zpool.tile([BC, 512], F32)
                    n0 = half * 512
                    for k in range(KC):
                        nc.tensor.matmul(
                            zp[:],
                            hT_prev[:, k * BC:(k + 1) * BC],
                            w_rhs[:, k * H + n0:k * H + n0 + 512],
                            start=(k == 0), stop=(k == KC - 1),
                        )
                    zs = s_z.tile([BC, 512], MDT)
                    if t_x is not None:
                        # z += x_t * wih, fused into the PSUM->SBUF move
                        nc.vector.scalar_tensor_tensor(
                            zs[:], wihb[:, n0:n0 + 512], xin[:, t_x:t_x + 1],
                            zp[:],
                            op0=mybir.AluOpType.mult, op1=mybir.AluOpType.add,
                        )
                    else:
                        nc.scalar.copy(zs[:], zp[:])
                    ztp = tpool.tile([128, 4 * BC], MDT)
                    for c in range(4):
                        nc.tensor.transpose(
                            ztp[:, c * BC:(c + 1) * BC],
                            zs[:, c * 128:(c + 1) * 128],
                            ident,
                        )
                    for c in range(4):
                        kchunk = half * 4 + c
                        nc.scalar.activation(
                            hT_new[:, kchunk * BC:(kchunk + 1) * BC],
                            ztp[:, c * BC:(c + 1) * BC],
                            mybir.ActivationFunctionType.Tanh,
                            bias=bias_col[:, kchunk:kchunk + 1],
                        )
                return hT_new

            # ---- encoder ----
            for t in range(t_enc):
                hT = step(hT, wet, bias_e, t_x=t)

            # ---- features = sigmoid(Wenc h_T + benc) ----
            fp = mpool.tile([BC, 4], F32, tag="m")
            for k in range(KC):
                nc.tensor.matmul(
                    fp[:],
                    hT[:, k * BC:(k + 1) * BC],
                    wenct[:, k * 4:(k + 1) * 4],
                    start=(k == 0), stop=False,
                )
            nc.tensor.matmul(fp[:], onesr, bencr, start=False, stop=True)
            feat = singles.tile([BC, 4], MDT)
            nc.scalar.activation(feat[:], fp[:],
                                 mybir.ActivationFunctionType.Sigmoid)
            nc.sync.dma_start(d_feat[:], feat[:, :3])

            # featT [4, BC] via PE transpose (row 3 is sigmoid(junk), but
            # wdect row 3 is zero so it does not contribute)
            ftp = mpool.tile([4, BC], MDT, tag="m")
            nc.tensor.transpose(ftp[:], feat[:], ident)
            featT = singles.tile([4, BC], MDT)
            nc.scalar.copy(featT[:], ftp[:])

            # ---- decoder h0 = Wdec features + bdec (no tanh) ----
            h0T = s_hT.tile([128, KC * BC], MDT)
            for half in range(2):
                zp = zpool.tile([BC, 512], F32)
                n0 = half * 512
                nc.tensor.matmul(zp[:], featT[:], wdect[:, n0:n0 + 512],
                                 start=True, stop=True)
                zs = s_z.tile([BC, 512], MDT)
                nc.scalar.copy(zs[:], zp[:])
                ztp = tpool.tile([128, 4 * BC], MDT)
                for c in range(4):
                    nc.tensor.transpose(ztp[:, c * BC:(c + 1) * BC],
                                        zs[:, c * 128:(c + 1) * 128], ident)
                for c in range(4):
                    kchunk = half * 4 + c
                    nc.scalar.activation(
                        h0T[:, kchunk * BC:(kchunk + 1) * BC],
                        ztp[:, c * BC:(c + 1) * BC],
                        mybir.ActivationFunctionType.Identity,
                        bias=bias_dec[:, kchunk:kchunk + 1],
                    )
            hT = h0T

            # ---- decoder scan; collect out_t = wout . h_t into PSUM ----
            xps = xpool.tile([BC, t_dec], F32)
            for t in range(1, t_dec + 1):
                if t == 1:
                    hT = step(hT, wdh, bias_d1)
                else:
                    hT = step(hT, wdt, bias_dt)
                for k in range(KC):
                    nc.tensor.matmul(
                        xps[:, t - 1:t],
                        hT[:, k * BC:(k + 1) * BC].bitcast(F32),
                        woutt[:, k:k + 1],
                        start=(k == 0), stop=(k == KC - 1),
                    )

            # ---- epilogue: output, masked-MSE partial ----
            out_sb = singles.tile([BC, t_dec], F32)
            nc.scalar.activation(out_sb[:], xps[:],
                                 mybir.ActivationFunctionType.Identity,
                                 bias=bout)
            nc.sync.dma_start(d_out[:], out_sb[:])

            diff = singles.tile([BC, t_dec], F32)
            nc.vector.tensor_sub(diff[:], xin[:, :t_dec], out_sb[:])
            d2 = singles.tile([BC, t_dec], F32)
            nc.vector.tensor_mul(d2[:], diff[:], diff[:])
            md = singles.tile([BC, t_dec], F32)
            nc.vector.scalar_tensor_tensor(
                md[:], iota, seqf, d2[:],
                op0=mybir.AluOpType.is_lt, op1=mybir.AluOpType.mult,
            )
            lp = singles.tile([BC, 1], F32)
            nc.vector.reduce_sum(lp[:], md[:], axis=mybir.AxisListType.X)
            nc.sync.dma_start(d_loss[:], lp[:])

    nc.compile()
    return nc


def host_prepare(inputs, t_enc=T, t_dec=T):
    """Shard + preprocess the full inputs into 8 per-core in_maps."""
    f = np.float32
    inp = np.asarray(inputs["input"], f)              # (B, T, 1)
    seq = np.asarray(inputs["seq_lengths"])           # (B,) int32
    whh_e = np.asarray(inputs["Whh_e"], f)
    wih_e = np.asarray(inputs["Wih_e"], f)[:, 0]      # (H,)
    b_e = (np.asarray(inputs["bih_e"], f) + np.asarray(inputs["bhh_e"], f))
    wenc = np.asarray(inputs["Wenc"], f)              # (3, H)
    benc = np.asarray(inputs["benc"], f)              # (3,)
    wdec = np.asarray(inputs["Wdec"], f)              # (H, 3)
    bdec = np.asarray(inputs["bdec"], f)              # (H,)
    whh_d = np.asarray(inputs["Whh_d"], f)
    wih_d = np.asarray(inputs["Wih_d"], f)[:, 0]      # (H,)
    b_d = (np.asarray(inputs["bih_d"], f) + np.asarray(inputs["bhh_d"], f))
    wout = np.asarray(inputs["Wout"], f)[0]           # (H,)
    bout = float(np.asarray(inputs["bout"], f)[0])

    wtil = whh_d + np.outer(wih_d, wout)              # x-feedback folded
    btil = b_d + bout * wih_d

    def slab(w):
        # rhs layout: [128, kchunk*H + n] = W^T[kchunk*128 + p, n]
        return np.ascontiguousarray(
            w.T.reshape(KC, 128, H).transpose(1, 0, 2).reshape(128, KC * H))

    def col(v):
        return np.ascontiguousarray(v.reshape(KC, 128).T)  # [128, KC]

    lay, total = blob_layout(t_enc, t_dec)
    blob = np.zeros((128, total), f)

    def put(name, arr):
        p, o, n = lay[name]
        assert arr.shape == (p, n), (name, arr.shape, (p, n))
        blob[:p, o:o + n] = arr

    put("wet", slab(whh_e))
    put("wdt", slab(wtil))
    put("wdh", slab(whh_d))
    wenct4 = np.zeros((KC, 128, 4), f)
    wenct4[:, :, :3] = wenc.T.reshape(KC, 128, 3)
    put("wenct", np.ascontiguousarray(
        wenct4.transpose(1, 0, 2).reshape(128, KC * 4)))
    put("woutt", col(wout))
    put("bias_e", col(b_e))
    put("bias_d1", col(b_d))
    put("bias_dt", col(btil))
    put("bias_dec", col(bdec))
    put("ident", np.eye(BC, dtype=f))
    wdect4 = np.zeros((4, H), f)
    wdect4[:3] = wdec.T
    put("wdect", wdect4)
    benc4 = np.zeros((1, 4), f)
    benc4[0, :3] = benc
    put("bencr", benc4)
    put("onesr", np.ones((1, BC), f))
    # h0 stays zeros
    put("wihb", np.broadcast_to(wih_e, (BC, H)))
    put("iota", np.broadcast_to(np.arange(t_dec, dtype=f), (BC, t_dec)))
    put("bout", np.full((BC, 1), bout, f))

    in_maps = []
    for c in range(NCORES):
        bl = blob.copy()
        p, o, n = lay["xin"]
        bl[:p, o:o + n] = inp[c * BC:(c + 1) * BC, :t_enc, 0]
        p, o, n = lay["seqf"]
        bl[:p, o:o + n] = seq[c * BC:(c + 1) * BC].astype(f)[:, None]
        in_maps.append({"blob": bl})
    return in_maps


_CACHE = {}


def kernel(**inputs):
    if "nc" not in _CACHE:
        _CACHE["nc"] = build()
    nc = _CACHE["nc"]
    in_maps = host_prepare(inputs)
    res = run_bass_kernel_spmd(nc, in_maps, core_ids=list(range(NCORES)))
    outs = res.results

    seq = np.asarray(inputs["seq_lengths"])
    output = np.concatenate([r["output"] for r in outs], 0)  # (B, T)
    feats = np.concatenate([r["features"] for r in outs], 0)  # (B, 3)
    lsum = float(sum(r["losspart"].sum() for r in outs))
    loss = np.float32(lsum / float(seq.sum()))
    inp = np.asarray(inputs["input"], np.float32)
    return (loss, inp, output[..., None].astype(np.float32), feats)


# revision 19
# speedup vs baseline: 3.1343x; 3.1343x over previous
"""Trainium2 Bass kernel for the RNN auto-encoder problem.

Model (per batch row b):
  encoder: h_t = tanh(x_t * wih_e + Whh_e h_{t-1} + b_e),  h_0 = 0,  t = 1..T
  features = sigmoid(Wenc h_T + benc)                       (3 dims)
  decoder: h0 = Wdec features + bdec
           h_1 = tanh(Whh_d h0 + b_d)
           h_t = tanh(Wt h_{t-1} + bt),  Wt = Whh_d + wih_d wout^T (x feedback folded)
           out_t = wout . h_t + bout
  loss = masked MSE(input, out)

Sharding: data parallel, B=256 -> 32 rows on each of 8 cores; weights replicated.

Device design (per core, B=32):
  state hT  : SBUF [128, 8*32]  (partition = H row % 128, free = chunk*32 + b)
  weights   : W^T as rhs slabs [128, 8*1024] (free = kchunk*1024 + n)
  per step  : 8 accumulating fp32r matmuls (lhsT = hT chunk [128,32]
              stationary, rhs = W^T slab [128,512]) per N-half -> PSUM
              z [32,512]; PSUM->SBUF move (encoder: fused with the rank-1
              x_t*wih input term via one DVE scalar_tensor_tensor); 4 PE
              transposes -> PSUM zT [128,128]; tanh fused into the
              PSUM->SBUF copy with per-partition (per-H-row) bias.
  decoder out_t collected via 8 N=1 matmuls into a persistent PSUM bank.

All host-prepared constants ship in ONE dram blob -> ONE DMA -> one
semaphore, because the fp32r matmul weight-load path tolerates only a
single sync wait per instruction.
"""

import numpy as np

import concourse.bacc as bacc
import concourse.bass as bass
import concourse.mybir as mybir
import concourse.tile as tile
from concourse.bass_utils import run_bass_kernel_spmd

B, T, H, I = 256, 512, 1024, 1
NCORES = 8
BC = B // NCORES          # 32 batch rows per core
KC = H // 128             # 8 contraction chunks
F32 = mybir.dt.float32
F32R = mybir.dt.float32r

USE_F32R = True           # fp32r matmuls: 1 cyc/row at N>=512 (vs 4 for fp32)
MDT = F32R if USE_F32R else F32


def blob_layout(t_enc, t_dec):
    """name -> (parts, free_off, free_len); everything lives in one
    [128, total] fp32 blob."""
    spec = [
        ("wet", 128, KC * H), ("wdt", 128, KC * H), ("wdh", 128, KC * H),
        ("wenct", 128, KC * 4), ("woutt", 128, KC),
        ("bias_e", 128, KC), ("bias_d1", 128, KC), ("bias_dt", 128, KC),
        ("bias_dec", 128, KC),
        ("ident", BC, BC), ("wdect", 4, H), ("bencr", 1, 4),
        ("onesr", 1, BC), ("h0", 128, KC * BC), ("wihb", BC, H),
        ("xin", BC, t_enc), ("iota", BC, t_dec), ("seqf", BC, 1),
        ("bout", BC, 1),
    ]
    out, off = {}, 0
    for name, p, n in spec:
        out[name] = (p, off, n)
        off += n
    return out, off


def build(t_enc=T, t_dec=T, efac=1, dfac=1, zb=False, pipe=0):
    """Build the per-core Bass kernel. All 8 cores run the same program."""
    nc = bacc.Bacc("TRN2", target_bir_lowering=False)
    lay, total = blob_layout(t_enc, t_dec)

    d_blob = nc.dram_tensor("blob", [128, total], MDT, kind="ExternalInput")
    d_out = nc.dram_tensor("output", [BC, t_dec], F32, kind="ExternalOutput")
    d_feat = nc.dram_tensor("features", [BC, 3], MDT, kind="ExternalOutput")
    d_loss = nc.dram_tensor("losspart", [BC, 1], F32, kind="ExternalOutput")

    with tile.TileContext(nc) as tc:
        with (
            tc.tile_pool(name="singles", bufs=1) as singles,
            tc.tile_pool(name="s_z", bufs=4) as s_z,
            tc.tile_pool(name="s_hT", bufs=4) as s_hT,
            tc.tile_pool(name="zpool", bufs=3, space="PSUM") as zpool,
            tc.tile_pool(name="tpool", bufs=3, space="PSUM") as tpool,
            tc.tile_pool(name="xpool", bufs=1, space="PSUM") as xpool,
            tc.tile_pool(name="mpool", bufs=1, space="PSUM") as mpool,
        ):
            blob = singles.tile([128, total], MDT)
            nc.sync.dma_start(blob[:], d_blob[:])

            def sl(name, dtype=None):
                p, o, n = lay[name]
                ap = blob[:p, o:o + n]
                return ap.bitcast(dtype) if dtype is not None else ap

            wet, wdt, wdh = sl("wet"), sl("wdt"), sl("wdh")
            wenct, woutt = sl("wenct"), sl("woutt", F32)
            bias_e, bias_d1 = sl("bias_e", F32), sl("bias_d1", F32)
            bias_dt, bias_dec = sl("bias_dt", F32), sl("bias_dec", F32)
            ident, wdect, bencr, onesr = (sl("ident"), sl("wdect"),
                                          sl("bencr"), sl("onesr"))
            wihb, xin = sl("wihb", F32), sl("xin", F32)
            iota, seqf, bout = sl("iota", F32), sl("seqf", F32), sl("bout", F32)

            hT = sl("h0")   # initial hidden state (zeros from host)

            def step(hT_prev, w_rhs, bias_col, t_x=None):
                """One RNN step -> new hT tile.

                z = W h (+ x_t * wih on encoder); h' = tanh(z + bias)
                """
                hT_new = s_hT.tile([128, KC * BC], MDT)
                for half in range(2):
                    zp = zpool.tile([BC, 512], F32)
                    n0 = half * 512
                    for k in range(KC):
                        nc.tensor.matmul(
                            zp[:],
                            hT_prev[:, k * BC:(k + 1) * BC],
                            w_rhs[:, k * H + n0:k * H + n0 + 512],
                            start=(k == 0), stop=(k == KC - 1),
                        )
                    zs = s_z.tile([BC, 512], MDT)
                    if t_x is not None:
                        # z += x_t * wih, fused into the PSUM->SBUF move
                        nc.vector.scalar_tensor_tensor(
                            zs[:], wihb[:, n0:n0 + 512], xin[:, t_x:t_x + 1],
                            zp[:],
                            op0=mybir.AluOpType.mult, op1=mybir.AluOpType.add,
                        )
                    elif zb:
                        nc.vector.tensor_copy(zs[:], zp[:])
                    else:
                        nc.scalar.copy(zs[:], zp[:])
                    ztp = tpool.tile([128, 4 * BC], MDT)
                    for c in range(4):
                        nc.tensor.transpose(
                            ztp[:, c * BC:(c + 1) * BC],
                            zs[:, c * 128:(c + 1) * 128],
                            ident,
                        )
                    h_out = hT_new[:, half * 4 * BC:(half + 1) * 4 * BC]
                    if zb:
                        nc.scalar.activation(
                            h_out, ztp[:], mybir.ActivationFunctionType.Tanh)
                    else:
                        for c in range(4):
                            kchunk = half * 4 + c
                            nc.scalar.activation(
                                hT_new[:, kchunk * BC:(kchunk + 1) * BC],
                                ztp[:, c * BC:(c + 1) * BC],
                                mybir.ActivationFunctionType.Tanh,
                                bias=bias_col[:, kchunk:kchunk + 1],
                            )
                return hT_new

            # -- software-pipelined emission (zero-bias path) --
            # The cross-step chain is z-MMs -> DVE move -> PE transpose ->
            # ACT tanh -> next z-MMs. Interleave each step's second-half
            # tail with the next step's matmuls so PE never waits.
            xps_ref = [None]

            def mm8(zp, hT_prev, w_rhs, n0, ks):
                for k in ks:
                    nc.tensor.matmul(
                        zp[:],
                        hT_prev[:, k * BC:(k + 1) * BC],
                        w_rhs[:, k * H + n0:k * H + n0 + 512],
                        start=(k == 0), stop=(k == KC - 1),
                        skip_group_check=True,
                    )

            def zmove(zs, zp, n0, t_x):
                if t_x is not None:
                    nc.vector.scalar_tensor_tensor(
                        zs[:], wihb[:, n0:n0 + 512], xin[:, t_x:t_x + 1],
                        zp[:],
                        op0=mybir.AluOpType.mult, op1=mybir.AluOpType.add)
                else:
                    nc.vector.tensor_copy(zs[:], zp[:])

            def tr4(ztp, zs):
                for c in range(4):
                    nc.tensor.transpose(ztp[:, c * BC:(c + 1) * BC],
                                        zs[:, c * 128:(c + 1) * 128], ident)

            def emit_tail(pend):
                zs1, hT_new, x_col = pend
                ztp1 = tpool.tile([128, 4 * BC], MDT, tag="ztp")
                tr4(ztp1, zs1)
                nc.scalar.activation(hT_new[:, 4 * BC:8 * BC], ztp1[:],
                                     mybir.ActivationFunctionType.Tanh)

            def emit_x(pend):
                _, hT_new, x_col = pend
                if x_col is not None:
                    for k in range(KC):
                        nc.tensor.matmul(
                            xps_ref[0][:, x_col:x_col + 1],
                            hT_new[:, k * BC:(k + 1) * BC].bitcast(F32),
                            woutt[:, k:k + 1],
                            start=(k == 0), stop=(k == KC - 1),
                            skip_group_check=True,
                        )

            def run_steps(hT, steps, pend):
                """steps: list of (w_rhs, t_x, x_col). Returns hT, pend."""
                for w_rhs, t_x, x_col in steps:
                    hT_prev = hT
                    hT_new = s_hT.tile([128, KC * BC], MDT, tag="hT")
                    if pipe == 2:
                        zp0 = zpool.tile([BC, 512], F32, tag="zp")
                        mm8(zp0, hT_prev, w_rhs, 0, range(0, 4))
                        if pend is not None:
                            emit_tail(pend)
                        zp1 = zpool.tile([BC, 512], F32, tag="zp")
                        mm8(zp1, hT_prev, w_rhs, 512, range(0, 8))
                        mm8(zp0, hT_prev, w_rhs, 0, range(4, 8))
                        zs0 = s_z.tile([BC, 512], MDT, tag="zs")
                        zmove(zs0, zp0, 0, t_x)
                        if pend is not None:
                            emit_x(pend)
                        ztp0 = tpool.tile([128, 4 * BC], MDT, tag="ztp")
                        tr4(ztp0, zs0)
                        nc.scalar.activation(hT_new[:, 0:4 * BC], ztp0[:],
                                             mybir.ActivationFunctionType.Tanh)
                        zs1 = s_z.tile([BC, 512], MDT, tag="zs")
                        zmove(zs1, zp1, 512, t_x)
                        pend = (zs1, hT_new, x_col)
                        hT = hT_new
                        continue
                    zp0 = zpool.tile([BC, 512], F32, tag="zp")
                    mm8(zp0, hT_prev, w_rhs, 0, range(0, 4))
                    if pend is not None:
                        emit_tail(pend)
                    zp1 = zpool.tile([BC, 512], F32, tag="zp")
                    mm8(zp1, hT_prev, w_rhs, 512, range(0, 4))
                    mm8(zp0, hT_prev, w_rhs, 0, range(4, 8))
                    zs0 = s_z.tile([BC, 512], MDT, tag="zs")
                    zmove(zs0, zp0, 0, t_x)
                    if pend is not None:
                        emit_x(pend)
                    mm8(zp1, hT_prev, w_rhs, 512, range(4, 8))
                    ztp0 = tpool.tile([128, 4 * BC], MDT, tag="ztp")
                    tr4(ztp0, zs0)
                    nc.scalar.activation(hT_new[:, 0:4 * BC], ztp0[:],
                                         mybir.ActivationFunctionType.Tanh)
                    zs1 = s_z.tile([BC, 512], MDT, tag="zs")
                    zmove(zs1, zp1, 512, t_x)
                    pend = (zs1, hT_new, x_col)
                    hT = hT_new
                return hT, pend

            def step3(hT_prev, w_rhs, t_x=None):
                """Sequential step, PSUM->SBUF move split so transposes
                start earlier (zero-bias only)."""
                hT_new = s_hT.tile([128, KC * BC], MDT)
                for half in range(2):
                    zp = zpool.tile([BC, 512], F32)
                    n0 = half * 512
                    for k in range(KC):
                        nc.tensor.matmul(
                            zp[:],
                            hT_prev[:, k * BC:(k + 1) * BC],
                            w_rhs[:, k * H + n0:k * H + n0 + 512],
                            start=(k == 0), stop=(k == KC - 1),
                        )
                    zs = s_z.tile([BC, 512], MDT)
                    for piece in range(2):
                        pl = piece * 256
                        if t_x is not None:
                            nc.vector.scalar_tensor_tensor(
                                zs[:, pl:pl + 256],
                                wihb[:, n0 + pl:n0 + pl + 256],
                                xin[:, t_x:t_x + 1], zp[:, pl:pl + 256],
                                op0=mybir.AluOpType.mult,
                                op1=mybir.AluOpType.add)
                        else:
                            nc.vector.tensor_copy(zs[:, pl:pl + 256],
                                                  zp[:, pl:pl + 256])
                        if piece == 0:
                            ztp = tpool.tile([128, 4 * BC], MDT, tag="ztp")
                        for c in (piece * 2, piece * 2 + 1):
                            nc.tensor.transpose(
                                ztp[:, c * BC:(c + 1) * BC],
                                zs[:, c * 128:(c + 1) * 128], ident)
                    h_out = hT_new[:, half * 4 * BC:(half + 1) * 4 * BC]
                    nc.scalar.activation(
                        h_out, ztp[:], mybir.ActivationFunctionType.Tanh)
                return hT_new

            # ---- encoder ----
            if zb and pipe == 3:
                for t in range(t_enc * efac):
                    hT = step3(hT, wet, t_x=t % t_enc)
            elif zb and pipe == 0:
                for t in range(t_enc * efac):
                    hT = step(hT, wet, bias_e, t_x=t % t_enc)
            elif zb:
                steps = [(wet, t % t_enc, None) for t in range(t_enc * efac)]
                hT, pend = run_steps(hT, steps, None)
                emit_tail(pend)
                emit_x(pend)
            else:
                for t in range(t_enc * efac):
                    hT = step(hT, wet, bias_e, t_x=t % t_enc)

            # ---- features = sigmoid(Wenc h_T + benc) ----
            fp = mpool.tile([BC, 4], F32, tag="m")
            for k in range(KC):
                nc.tensor.matmul(
                    fp[:],
                    hT[:, k * BC:(k + 1) * BC],
                    wenct[:, k * 4:(k + 1) * 4],
                    start=(k == 0), stop=False,
                )
            nc.tensor.matmul(fp[:], onesr, bencr, start=False, stop=True)
            feat = singles.tile([BC, 4], MDT)
            nc.scalar.activation(feat[:], fp[:],
                                 mybir.ActivationFunctionType.Sigmoid)
            nc.sync.dma_start(d_feat[:], feat[:, :3])

            # featT [4, BC] via PE transpose (row 3 is sigmoid(junk), but
            # wdect row 3 is zero so it does not contribute)
            ftp = mpool.tile([4, BC], MDT, tag="m")
            nc.tensor.transpose(ftp[:], feat[:], ident)
            featT = singles.tile([4, BC], MDT)
            nc.scalar.copy(featT[:], ftp[:])

            # ---- decoder h0 = Wdec features + bdec (no tanh) ----
            h0T = s_hT.tile([128, KC * BC], MDT)
            for half in range(2):
                zp = zpool.tile([BC, 512], F32)
                n0 = half * 512
                nc.tensor.matmul(zp[:], featT[:], wdect[:, n0:n0 + 512],
                                 start=True, stop=True)
                zs = s_z.tile([BC, 512], MDT)
                nc.scalar.copy(zs[:], zp[:])
                ztp = tpool.tile([128, 4 * BC], MDT)
                for c in range(4):
                    nc.tensor.transpose(ztp[:, c * BC:(c + 1) * BC],
                                        zs[:, c * 128:(c + 1) * 128], ident)
                if zb:
                    nc.scalar.copy(
                        h0T[:, half * 4 * BC:(half + 1) * 4 * BC], ztp[:])
                else:
                    for c in range(4):
                        kchunk = half * 4 + c
                        nc.scalar.activation(
                            h0T[:, kchunk * BC:(kchunk + 1) * BC],
                            ztp[:, c * BC:(c + 1) * BC],
                            mybir.ActivationFunctionType.Identity,
                            bias=bias_dec[:, kchunk:kchunk + 1],
                        )
            hT = h0T

            # ---- decoder scan; collect out_t = wout . h_t into PSUM ----
            xps = xpool.tile([BC, t_dec], F32)
            xps_ref[0] = xps
            if zb and pipe in (0, 3):
                for t in range(1, t_dec * dfac + 1):
                    hT = (step3(hT, wdh) if pipe == 3 else
                          step(hT, wdh, bias_d1)) if t == 1 else (
                          step3(hT, wdt) if pipe == 3 else
                          step(hT, wdt, bias_dt))
                    col = (t - 1) % t_dec
                    for k in range(KC):
                        nc.tensor.matmul(
                            xps[:, col:col + 1],
                            hT[:, k * BC:(k + 1) * BC].bitcast(F32),
                            woutt[:, k:k + 1],
                            start=(k == 0), stop=(k == KC - 1),
                        )
            elif zb:
                steps = [(wdh if t == 1 else wdt, None, (t - 1) % t_dec)
                         for t in range(1, t_dec * dfac + 1)]
                hT, pend = run_steps(hT, steps, None)
                emit_tail(pend)
                emit_x(pend)
            else:
                for t in range(1, t_dec * dfac + 1):
                    if t == 1:
                        hT = step(hT, wdh, bias_d1)
                    else:
                        hT = step(hT, wdt, bias_dt)
                    col = (t - 1) % t_dec
                    for k in range(KC):
                        nc.tensor.matmul(
                            xps[:, col:col + 1],
                            hT[:, k * BC:(k + 1) * BC].bitcast(F32),
                            woutt[:, k:k + 1],
                            start=(k == 0), stop=(k == KC - 1),
                        )

            # ---- epilogue: output, masked-MSE partial ----
            out_sb = singles.tile([BC, t_dec], F32)
            nc.scalar.activation(out_sb[:], xps[:],
                                 mybir.ActivationFunctionType.Identity,
                                 bias=bout)
            nc.sync.dma_start(d_out[:], out_sb[:])

            diff = singles.tile([BC, t_dec], F32)
            nc.vector.tensor_sub(diff[:], xin[:, :t_dec], out_sb[:])
            d2 = singles.tile([BC, t_dec], F32)
            nc.vector.tensor_mul(d2[:], diff[:], diff[:])
            md = singles.tile([BC, t_dec], F32)
            nc.vector.scalar_tensor_tensor(
                md[:], iota, seqf, d2[:],
                op0=mybir.AluOpType.is_lt, op1=mybir.AluOpType.mult,
            )
            lp = singles.tile([BC, 1], F32)
            nc.vector.reduce_sum(lp[:], md[:], axis=mybir.AxisListType.X)
            nc.sync.dma_start(d_loss[:], lp[:])

    nc.compile()
    return nc


def host_prepare(inputs, t_enc=T, t_dec=T):
    """Shard + preprocess the full inputs into 8 per-core in_maps."""
    f = np.float32
    inp = np.asarray(inputs["input"], f)              # (B, T, 1)
    seq = np.asarray(inputs["seq_lengths"])           # (B,) int32
    whh_e = np.asarray(inputs["Whh_e"], f)
    wih_e = np.asarray(inputs["Wih_e"], f)[:, 0]      # (H,)
    b_e = (np.asarray(inputs["bih_e"], f) + np.asarray(inputs["bhh_e"], f))
    wenc = np.asarray(inputs["Wenc"], f)              # (3, H)
    benc = np.asarray(inputs["benc"], f)              # (3,)
    wdec = np.asarray(inputs["Wdec"], f)              # (H, 3)
    bdec = np.asarray(inputs["bdec"], f)              # (H,)
    whh_d = np.asarray(inputs["Whh_d"], f)
    wih_d = np.asarray(inputs["Wih_d"], f)[:, 0]      # (H,)
    b_d = (np.asarray(inputs["bih_d"], f) + np.asarray(inputs["bhh_d"], f))
    wout = np.asarray(inputs["Wout"], f)[0]           # (H,)
    bout = float(np.asarray(inputs["bout"], f)[0])

    wtil = whh_d + np.outer(wih_d, wout)              # x-feedback folded
    btil = b_d + bout * wih_d

    def slab(w):
        # rhs layout: [128, kchunk*H + n] = W^T[kchunk*128 + p, n]
        return np.ascontiguousarray(
            w.T.reshape(KC, 128, H).transpose(1, 0, 2).reshape(128, KC * H))

    def col(v):
        return np.ascontiguousarray(v.reshape(KC, 128).T)  # [128, KC]

    lay, total = blob_layout(t_enc, t_dec)
    blob = np.zeros((128, total), f)

    def put(name, arr):
        p, o, n = lay[name]
        assert arr.shape == (p, n), (name, arr.shape, (p, n))
        blob[:p, o:o + n] = arr

    put("wet", slab(whh_e))
    put("wdt", slab(wtil))
    put("wdh", slab(whh_d))
    wenct4 = np.zeros((KC, 128, 4), f)
    wenct4[:, :, :3] = wenc.T.reshape(KC, 128, 3)
    put("wenct", np.ascontiguousarray(
        wenct4.transpose(1, 0, 2).reshape(128, KC * 4)))
    put("woutt", col(wout))
    put("bias_e", col(b_e))
    put("bias_d1", col(b_d))
    put("bias_dt", col(btil))
    put("bias_dec", col(bdec))
    put("ident", np.eye(BC, dtype=f))
    wdect4 = np.zeros((4, H), f)
    wdect4[:3] = wdec.T
    put("wdect", wdect4)
    benc4 = np.zeros((1, 4), f)
    benc4[0, :3] = benc
    put("bencr", benc4)
    put("onesr", np.ones((1, BC), f))
    # h0 stays zeros
    put("wihb", np.broadcast_to(wih_e, (BC, H)))
    put("iota", np.broadcast_to(np.arange(t_dec, dtype=f), (BC, t_dec)))
    put("bout", np.full((BC, 1), bout, f))

    in_maps = []
    for c in range(NCORES):
        bl = blob.copy()
        p, o, n = lay["xin"]
        bl[:p, o:o + n] = inp[c * BC:(c + 1) * BC, :t_enc, 0]
        p, o, n = lay["seqf"]
        bl[:p, o:o + n] = seq[c * BC:(c + 1) * BC].astype(f)[:, None]
        in_maps.append({"blob": bl})
    return in_maps


_CACHE = {}


def _all_bias_zero(inputs):
    names = ["bih_e", "bhh_e", "bih_d", "bhh_d", "bdec", "bout"]
    vals = [np.asarray(inputs[n]) if n != "bdec" else np.asarray(inputs["bdec"])
            for n in names]
    return all(not np.any(v) for v in vals)


def kernel(**inputs):
    zb = _all_bias_zero(inputs)
    key = ("nc", zb)
    if key not in _CACHE:
        _CACHE[key] = build(zb=zb)
    nc = _CACHE[key]
    in_maps = host_prepare(inputs)
    res = run_bass_kernel_spmd(nc, in_maps, core_ids=list(range(NCORES)))
    outs = res.results

    seq = np.asarray(inputs["seq_lengths"])
    output = np.concatenate([r["output"] for r in outs], 0)  # (B, T)
    feats = np.concatenate([r["features"] for r in outs], 0)  # (B, 3)
    lsum = float(sum(r["losspart"].sum() for r in outs))
    loss = np.float32(lsum / float(seq.sum()))
    inp = np.asarray(inputs["input"], np.float32)
    return (loss, inp, output[..., None].astype(np.float32), feats)
